# revision 10
# baseline (speedup 1.0000x reference)
"""Multi-head causal attention (B=4,T=2048,C=1024,H=16,D=64) on 8 TRN2 NeuronCores.

Sharding: no collectives. Core c handles batch b=c//2 and a causally-balanced
set of four 256-query chunks (half=c%2): half0 -> chunks [0,2,5,7], half1 ->
[1,3,4,6], processed in slot order with padded per-slot key-tile counts
[4,8,12,16]. Every core runs the same SPMD program; per-core differences are
expressed purely through input data:
  - half1 cores get xT with the two 256-token blocks of each 512-token stripe
    swapped, which makes the query-column offset of the slot-s chunk inside
    stripe s uniform ([0,0,256,256]) across halves, and
  - per-half multiplicative {0,1} causal/pad masks for the last 4 key tiles of
    each slot (key order follows the same permutation; attention is
    key-order-invariant).

All-bf16 (inputs converted on host), K/V SBUF-resident (no DRAM scratch),
causal masks as 0/1 DVE multiplies, x streamed per 512-token stripe.

Schedule (one Bass/Tile program): the C-stage of slot s-1 is interleaved
between the K-projection groups of stripe s so the in-order PE queue always
has independent projection matmuls between exp-dependent score/AV matmuls.
Slot 3's key-tile loop is split: tiles 0-11 (whose keys only need stripes
0-2) run inside the stripe-3 iteration with their partial AV staged to SBUF
bf16; tiles 12-15 plus the merge/normalize run in the tail, which shrinks the
ACT(exp)-bound tail from ~16 to ~4 key tiles per pair:
  s=0: x0 DMA, Q(0), K(0), V(0)
  s=1,2: xs DMA, Q(s), [C(s-1,p) | K(s,p) for p], V(s), proj(s-2)
  s=3:   x3 DMA, Q(3), [C(2,p) | K(3,p) | C(3,p) tiles 0-11 for p], V(3), proj(1)
  tail: proj(2), [C(3,p) tiles 12-15 + merge + norm for p], proj(3)
C(slot): per head-pair p, per key-tile pair: scores = kT^T @ qT (two heads
packed via PE row groups, separate PSUM banks), exp on ACT (scale folded),
0/1 mask multiplies on DVE for the last 4 key tiles, wei @ [v|1] accumulated
in PSUM ([65,512] per pair: row 64 = sumexp; one start/stop per bank).
Normalization is decoupled: avp is staged to SBUF bf16 (frees the PSUM slot),
then DVE fast reciprocal + GPSIMD partition broadcast + DVE multiplies.
"""

import numpy as np
import ml_dtypes

import concourse.bass as bass
import concourse.tile as tile
from concourse import bacc, library_config, mybir
from concourse.bass_utils import run_bass_kernel_spmd

B, T, C = 4, 2048, 1024
H, D = 16, 64
P = 128            # key tile size
QC = 256           # query chunk size
NP = 8             # head pairs
PNS = [4, 8, 12, 16]                     # padded per-slot key-tile counts
SLOT_CHUNKS = [[0, 2, 5, 7], [1, 3, 4, 6]]  # chunk ids per half, slot order
QOFF = [0, 0, 256, 256]  # query-col offset of slot-s chunk inside stripe s
BF16 = mybir.dt.bfloat16
F32 = mybir.dt.float32
EXP = mybir.ActivationFunctionType.Exp
SCALE = float(C) ** -0.5
VW = 130           # v cols per pair: [vA(64) | 1 | vB(64) | 1]
BD = ml_dtypes.bfloat16


def build_kernel(nc: bass.Bass):
    xT = nc.dram_tensor("xT", [C, T], BF16, kind="ExternalInput").ap()
    wq2 = nc.dram_tensor("wq2", [C, C], BF16, kind="ExternalInput").ap()
    wk2 = nc.dram_tensor("wk2", [C, C], BF16, kind="ExternalInput").ap()
    wv2 = nc.dram_tensor("wv2", [C, C], BF16, kind="ExternalInput").ap()
    wp = nc.dram_tensor("wp", [C, C], BF16, kind="ExternalInput").ap()
    bias2 = nc.dram_tensor("bias2", [P, C], BF16, kind="ExternalInput").ap()
    masks = nc.dram_tensor("masks", [P, 16 * QC], BF16, kind="ExternalInput").ap()
    out = nc.dram_tensor("out", [4, QC, C], BF16, kind="ExternalOutput").ap()

    with tile.TileContext(nc) as tc:
        nc.gpsimd.load_library(library_config.attn)
        with (
            tc.tile_pool(name="const", bufs=1) as cpool,
            tc.tile_pool(name="xs", bufs=2) as xpool,
            tc.tile_pool(name="exp", bufs=3) as epool,
            tc.tile_pool(name="outp", bufs=2) as opool,
            tc.tile_pool(name="norm", bufs=1) as npool,
            tc.tile_pool(name="ps", bufs=2, space="PSUM") as psp,
        ):
            wq_sb = cpool.tile([P, 8 * C], BF16)
            wk_sb = cpool.tile([P, 8 * C], BF16)
            wv_sb = cpool.tile([P, 8 * C], BF16)
            wp_sb = cpool.tile([P, 8 * C], BF16)
            qT_sb = cpool.tile([P, NP * 1024], BF16)
            kT_sb = cpool.tile([P, NP * T], BF16)
            v_sb = cpool.tile([P, 16 * NP * VW], BF16)
            masks_sb = cpool.tile([P, 16 * QC], BF16)
            bias_bc = cpool.tile([P, C], BF16)

            attn_tiles = {}

            def get_attn(k):
                # per-slot attention buffer [128, p*256+t], ring of 2
                if k not in attn_tiles:
                    attn_tiles[k] = cpool.tile(
                        [P, NP * QC], BF16, tag="at", bufs=2, name=f"attn{k}"
                    )
                return attn_tiles[k]

            def dma_w(dst, src):
                # whole [C, C] weight -> [128, 8*C] SBUF in one DMA
                nc.sync.dma_start(
                    dst[:].rearrange("p (g c) -> p g c", c=C),
                    src.rearrange("(g p) c -> p g c", p=P),
                )

            def dma_x(xs, s):
                nc.sync.dma_start(
                    xs[:].rearrange("p (g c) -> p g c", c=512),
                    xT.rearrange("(g p) t -> p g t", p=P)[
                        :, :, s * 512:(s + 1) * 512],
                )

            def q_stage(s, xs):
                for p in range(NP):
                    qp = psp.tile([P, QC], F32, tag="w2", name=f"qp{s}_{p}")
                    for g in range(8):
                        nc.tensor.matmul(
                            qp[:],
                            wq_sb[:, g * C + p * P:][:, :P],
                            xs[:, g * 512 + QOFF[s]:][:, :QC],
                            start=(g == 0), stop=(g == 7),
                        )
                    nc.scalar.copy(qT_sb[:, p * 1024 + s * QC:][:, :QC], qp[:])

            def k_group(s, xs, p):
                kp = psp.tile([P, 512], F32, tag="w2", name=f"kp{s}_{p}")
                for g in range(8):
                    nc.tensor.matmul(
                        kp[:],
                        wk_sb[:, g * C + p * P:][:, :P],
                        xs[:, g * 512:(g + 1) * 512],
                        start=(g == 0), stop=(g == 7),
                    )
                nc.vector.tensor_copy(kT_sb[:, p * T + s * 512:][:, :512], kp[:])

            def v_stage(s, xs):
                for jj in range(4):
                    j = 4 * s + jj
                    for hc in range(2):
                        vp = psp.tile([P, 512], F32, tag="w2",
                                      name=f"vp{j}_{hc}")
                        for g in range(8):
                            nc.tensor.matmul(
                                vp[:],
                                xs[:, g * 512 + jj * P:][:, :P],
                                wv_sb[:, g * C + hc * 512:][:, :512],
                                start=(g == 0), stop=(g == 7),
                            )
                        vdst = v_sb[:, j * (NP * VW) + hc * 4 * VW:][:, :4 * VW]
                        v3 = vdst.rearrange("p (l c) -> p l c", c=VW)
                        s3 = vp[:].rearrange("p (l c) -> p l c", c=P)
                        nc.scalar.copy(v3[:, :, 0:64], s3[:, :, 0:64])
                        nc.scalar.copy(v3[:, :, 65:129], s3[:, :, 64:128])

            def c_part(k, p, u_lo, u_hi, name):
                """Score/exp/mask/AV for key-tile pairs u_lo..u_hi-1 of
                (slot k, pair p). Returns the PSUM accumulator (one bank:
                one start=True on the first matmul, stop=True on the last
                of THIS part; interior matmuls overwrite-where-unset)."""
                pn = PNS[k]
                avp = psp.tile([65, 512], F32, tag="w2", name=f"av{name}")
                qA = qT_sb[0:64, p * 1024 + k * QC:][:, :QC]
                qB = qT_sb[64:128, p * 1024 + k * QC:][:, :QC]
                pend = None

                def emit_av(pv):
                    e_t, j0 = pv
                    first = j0 == 2 * u_lo
                    last = j0 + 2 == 2 * u_hi
                    b0 = j0 * (NP * VW) + p * VW
                    b1 = (j0 + 1) * (NP * VW) + p * VW
                    nc.tensor.matmul(avp[:, 0:QC], v_sb[:, b0:b0 + 65],
                                     e_t[:, 0:QC], start=first, stop=False)
                    nc.tensor.matmul(avp[:, 0:QC], v_sb[:, b1:b1 + 65],
                                     e_t[:, QC:2 * QC],
                                     start=False, stop=False)
                    nc.tensor.matmul(avp[:, QC:2 * QC], v_sb[:, b0 + 65:b0 + VW],
                                     e_t[:, 2 * QC:3 * QC],
                                     start=False, stop=False)
                    nc.tensor.matmul(avp[:, QC:2 * QC], v_sb[:, b1 + 65:b1 + VW],
                                     e_t[:, 3 * QC:4 * QC],
                                     start=False, stop=last)

                for u in range(u_lo, u_hi):
                    j0 = 2 * u
                    kt0 = kT_sb[:, p * T + j0 * P:][:, :P]
                    kt1 = kT_sb[:, p * T + (j0 + 1) * P:][:, :P]
                    # sc spans 2 PSUM banks (cols 0:512 / 512:1024): one
                    # start=True per bank; the second matmul into a bank
                    # runs accumulate-mode and overwrites its untouched half.
                    sc = psp.tile([P, 4 * QC], F32, tag="sc", bufs=3,
                                  name=f"sc{name}_{u}")
                    nc.tensor.matmul(sc[:, 0:QC], kt0[0:64, :], qA,
                                     start=True, stop=False,
                                     tile_position=(0, 0))
                    nc.tensor.matmul(sc[:, 2 * QC:3 * QC], kt0[64:128, :], qB,
                                     start=True, stop=False,
                                     tile_position=(64, 0))
                    nc.tensor.matmul(sc[:, QC:2 * QC], kt1[0:64, :], qA,
                                     start=False, stop=True,
                                     tile_position=(0, 0))
                    nc.tensor.matmul(sc[:, 3 * QC:4 * QC], kt1[64:128, :], qB,
                                     start=False, stop=True,
                                     tile_position=(64, 0))
                    e_t = epool.tile([P, 4 * QC], BF16, tag="e",
                                     name=f"e{name}_{u}")
                    nc.scalar.activation(e_t[:], sc[:], EXP, scale=SCALE)
                    if u >= pn // 2 - 2:
                        l0 = j0 - (pn - 4)
                        mi = (k * 4 + l0) * QC
                        m2 = masks_sb[:, mi:mi + 2 * QC]
                        nc.vector.tensor_mul(e_t[:, 0:2 * QC],
                                             e_t[:, 0:2 * QC], m2)
                        nc.vector.tensor_mul(e_t[:, 2 * QC:4 * QC],
                                             e_t[:, 2 * QC:4 * QC], m2)
                    if pend is not None:
                        emit_av(pend)
                    pend = (e_t, j0)
                emit_av(pend)
                return avp

            def c_norm(k, p, avst):
                """Normalize staged [65, 512] bf16 AV into the attn ring."""
                rs = npool.tile([1, 2 * QC], F32, tag="rs", name=f"rs{k}_{p}")
                nc.vector.tensor_copy(rs[:], avst[64:65, :])
                rc = npool.tile([1, 2 * QC], F32, tag="rc", name=f"rc{k}_{p}")
                nc.vector.reciprocal_approx_fast(rc[:], rs[:])
                rb = npool.tile([64, 2 * QC], F32, tag="rb", name=f"rb{k}_{p}")
                nc.gpsimd.partition_broadcast(rb[:], rc[:])
                attn_r = get_attn(k)
                col = p * QC
                nc.vector.tensor_mul(attn_r[0:64, col:col + QC],
                                     avst[0:64, 0:QC], rb[:, 0:QC])
                nc.vector.tensor_mul(attn_r[64:128, col:col + QC],
                                     avst[0:64, QC:2 * QC], rb[:, QC:2 * QC])

            def c_run(k, p):
                avp = c_part(k, p, 0, PNS[k] // 2, f"{k}_{p}")
                avst = npool.tile([65, 512], BF16, tag="avst", bufs=2,
                                  name=f"avst{k}_{p}")
                nc.vector.tensor_copy(avst[:], avp[:])
                c_norm(k, p, avst)

            def proj(k):
                attn_r = get_attn(k)
                for tt in range(2):
                    for oc in range(2):
                        pp = psp.tile([P, 512], F32, tag="w2",
                                      name=f"pp{k}_{tt}_{oc}")
                        for g in range(NP):
                            nc.tensor.matmul(
                                pp[:],
                                attn_r[:, g * QC + tt * P:][:, :P],
                                wp_sb[:, g * C + oc * 512:][:, :512],
                                start=(g == 0), stop=(g == 7),
                            )
                        ot = opool.tile([P, 512], BF16, tag="ot",
                                        name=f"ot{k}_{tt}_{oc}")
                        nc.vector.tensor_add(
                            ot[:], pp[:], bias_bc[:, oc * 512:(oc + 1) * 512]
                        )
                        nc.sync.dma_start(
                            out[k, tt * P:(tt + 1) * P, oc * 512:(oc + 1) * 512],
                            ot[:],
                        )

            # startup: interleave x-stripe-0 and wq per-g DMAs so Q(0) can
            # begin after the first blocks land; bulk weights follow.
            xs0 = xpool.tile([P, 8 * 512], BF16, tag="xs", name="xs0")
            for g in range(8):
                nc.sync.dma_start(
                    xs0[:, g * 512:(g + 1) * 512],
                    xT[g * P:(g + 1) * P, 0:512],
                )
                nc.sync.dma_start(
                    wq_sb[:, g * C:(g + 1) * C], wq2[g * P:(g + 1) * P, :]
                )
            dma_w(wk_sb, wk2)
            dma_w(wv_sb, wv2)
            q_stage(0, xs0)
            for p in range(NP):
                k_group(0, xs0, p)
            dma_w(wp_sb, wp)
            nc.sync.dma_start(masks_sb[:], masks[:])
            nc.sync.dma_start(bias_bc[:], bias2[:])
            # ones columns of v (col = 65*m + 64 for m in 0..255)
            vones = v_sb[:].rearrange("p (m o) -> p m o", o=65)[:, :, 64:65]
            nc.vector.memset(vones, 1.0)
            v_stage(0, xs0)

            av1 = {}
            for s in range(1, 4):
                xs = xpool.tile([P, 8 * 512], BF16, tag="xs", name=f"xs{s}")
                dma_x(xs, s)
                q_stage(s, xs)
                for p in range(NP):
                    c_run(s - 1, p)
                    k_group(s, xs, p)
                    if s == 3:
                        # slot 3, key tiles 0-11 (keys from stripes 0-2)
                        avp1 = c_part(3, p, 0, 6, f"3a_{p}")
                        av1[p] = npool.tile([65, 512], BF16, tag="av1",
                                            bufs=8, name=f"av1_{p}")
                        nc.vector.tensor_copy(av1[p][:], avp1[:])
                v_stage(s, xs)
                if s >= 2:
                    proj(s - 2)
            proj(2)
            for p in range(NP):
                # slot 3, key tiles 12-15 + merge with staged partial + norm
                avp2 = c_part(3, p, 6, 8, f"3b_{p}")
                avst = npool.tile([65, 512], BF16, tag="avst", bufs=2,
                                  name=f"avst3_{p}")
                nc.vector.tensor_add(avst[:], avp2[:], av1[p][:])
                c_norm(3, p, avst)
            proj(3)
    return nc


def _make_masks(half):
    m = np.zeros((P, 16 * QC), np.float32)
    s = np.arange(P)[:, None]
    t = np.arange(QC)[None, :]
    for k in range(4):
        q = SLOT_CHUNKS[half][k]
        pn = PNS[k]
        for l in range(4):
            j = pn - 4 + l
            a = j if half == 0 else 4 * (j // 4) + (j + 2) % 4
            m[:, (k * 4 + l) * QC:(k * 4 + l + 1) * QC] = (
                a * P + s <= q * QC + t
            )
    return m.astype(BD)


_CACHE = {}


def _get_nc():
    if "nc" not in _CACHE:
        nc = bacc.Bacc("TRN2", target_bir_lowering=False, debug=False)
        build_kernel(nc)
        nc.compile()
        _CACHE["nc"] = nc
    return _CACHE["nc"]


def make_in_maps(x, wq, wk, wv, w_proj, b_proj):
    x = np.asarray(x, np.float32)
    wq2 = np.ascontiguousarray(
        np.transpose(np.asarray(wq), (1, 0, 2)).reshape(C, C)).astype(BD)
    wk2 = np.ascontiguousarray(
        np.transpose(np.asarray(wk), (1, 0, 2)).reshape(C, C)).astype(BD)
    wv2 = np.ascontiguousarray(
        np.transpose(np.asarray(wv), (1, 0, 2)).reshape(C, C)).astype(BD)
    wpm = np.asarray(w_proj, np.float32).astype(BD)
    bias2 = np.tile(np.asarray(b_proj, np.float32).reshape(1, C), (P, 1))
    bias2 = np.ascontiguousarray(bias2).astype(BD)
    masks_h = [_make_masks(0), _make_masks(1)]

    in_maps = []
    for core in range(8):
        b, half = core // 2, core % 2
        xb = x[b]
        if half == 1:
            # swap the two 256-blocks of each 512-token stripe
            xb = xb.reshape(4, 2, QC, C)[:, ::-1].reshape(T, C)
        xTb = np.ascontiguousarray(xb.T).astype(BD)
        in_maps.append({
            "xT": xTb,
            "wq2": wq2, "wk2": wk2, "wv2": wv2,
            "wp": wpm, "bias2": bias2, "masks": masks_h[half],
        })
    return in_maps


def assemble(results):
    full = np.zeros((B, T, C), np.float32)
    for core in range(8):
        b, half = core // 2, core % 2
        o = np.asarray(results[core]["out"], dtype=np.float32)
        for k, q in enumerate(SLOT_CHUNKS[half]):
            full[b, q * QC:(q + 1) * QC] = o[k]
    return full


def kernel(x, wq, wk, wv, w_proj, b_proj, _trace=False, _tmpdir=None):
    in_maps = make_in_maps(x, wq, wk, wv, w_proj, b_proj)
    nc = _get_nc()
    res = run_bass_kernel_spmd(
        nc, in_maps, core_ids=list(range(8)), trace=_trace, tmpdir=_tmpdir
    )
    if _trace:
        _CACHE["last_result"] = res
    return assemble(res.results)


# revision 11
# speedup vs baseline: 1.0490x; 1.0490x over previous
"""Multi-head causal attention (B=4,T=2048,C=1024,H=16,D=64) on 8 TRN2 NeuronCores.

Sharding: no collectives. Core c handles batch b=c//2 and a causally-balanced
set of four 256-query chunks (half=c%2): half0 -> chunks [0,2,5,7], half1 ->
[1,3,4,6], processed in slot order with padded per-slot key-tile counts
[4,8,12,16]. Every core runs the same SPMD program; per-core differences are
expressed purely through input data:
  - half1 cores get xT with the two 256-token blocks of each 512-token stripe
    swapped, which makes the query-column offset of the slot-s chunk inside
    stripe s uniform ([0,0,256,256]) across halves, and
  - per-half multiplicative {0,1} causal/pad masks for the last 4 key tiles of
    each slot (key order follows the same permutation; attention is
    key-order-invariant).

All-bf16 (inputs converted on host), K/V SBUF-resident (no DRAM scratch),
causal masks as 0/1 DVE multiplies, x streamed per 512-token stripe.

Schedule (one Bass/Tile program): the C-stage of slot s-1 is interleaved
between the K-projection groups of stripe s so the in-order PE queue always
has independent projection matmuls between exp-dependent score/AV matmuls.
Slot 3's work is spread across the whole timeline to keep the ACT(exp) load
level: its query columns are prefetched so Q(3) runs at stripe 2, key tiles
0-7 run inside stripe 2, tiles 8-11 inside stripe 3 (partials accumulated in
SBUF bf16), and only tiles 12-15 + merge + normalize run in the tail:
  s=0: x0 DMA, Q(0), K(0), V(0)
  s=1:   x1 DMA, Q(1), [C(0,p) | K(1,p) for p], V(1)
  s=2:   x2 DMA, Q(2), Q(3), [C(1,p) | K(2,p) | C3a(p: tiles 0-7) for p],
         V(2), proj(0)
  s=3:   x3 DMA, [C(2,p) | K(3,p) | C3b(p: tiles 8-11) for p], V(3), proj(1)
  tail: proj(2), [C3c(p: tiles 12-15) + merge + norm for p], proj(3)
C(slot): per head-pair p, per key-tile pair: scores = kT^T @ qT (two heads
packed via PE row groups, separate PSUM banks), exp on ACT (scale folded),
0/1 mask multiplies on DVE for the last 4 key tiles, wei @ [v|1] accumulated
in PSUM ([65,512] per pair: row 64 = sumexp; one start/stop per bank).
Normalization is decoupled: avp is staged to SBUF bf16 (frees the PSUM slot),
then DVE fast reciprocal + GPSIMD partition broadcast + DVE multiplies.
"""

import numpy as np
import ml_dtypes

import concourse.bass as bass
import concourse.tile as tile
from concourse import bacc, library_config, mybir
from concourse.bass_utils import run_bass_kernel_spmd

B, T, C = 4, 2048, 1024
H, D = 16, 64
P = 128            # key tile size
QC = 256           # query chunk size
NP = 8             # head pairs
PNS = [4, 8, 12, 16]                     # padded per-slot key-tile counts
SLOT_CHUNKS = [[0, 2, 5, 7], [1, 3, 4, 6]]  # chunk ids per half, slot order
QOFF = [0, 0, 256, 256]  # query-col offset of slot-s chunk inside stripe s
BF16 = mybir.dt.bfloat16
F32 = mybir.dt.float32
EXP = mybir.ActivationFunctionType.Exp
SCALE = float(C) ** -0.5
VW = 130           # v cols per pair: [vA(64) | 1 | vB(64) | 1]
BD = ml_dtypes.bfloat16


def build_kernel(nc: bass.Bass):
    xT = nc.dram_tensor("xT", [C, T], BF16, kind="ExternalInput").ap()
    wq2 = nc.dram_tensor("wq2", [C, C], BF16, kind="ExternalInput").ap()
    wk2 = nc.dram_tensor("wk2", [C, C], BF16, kind="ExternalInput").ap()
    wv2 = nc.dram_tensor("wv2", [C, C], BF16, kind="ExternalInput").ap()
    wp = nc.dram_tensor("wp", [C, C], BF16, kind="ExternalInput").ap()
    bias2 = nc.dram_tensor("bias2", [P, C], BF16, kind="ExternalInput").ap()
    masks = nc.dram_tensor("masks", [P, 16 * QC], BF16, kind="ExternalInput").ap()
    out = nc.dram_tensor("out", [4, QC, C], BF16, kind="ExternalOutput").ap()

    with tile.TileContext(nc) as tc:
        nc.gpsimd.load_library(library_config.attn)
        with (
            tc.tile_pool(name="const", bufs=1) as cpool,
            tc.tile_pool(name="xs", bufs=2) as xpool,
            tc.tile_pool(name="exp", bufs=3) as epool,
            tc.tile_pool(name="outp", bufs=2) as opool,
            tc.tile_pool(name="norm", bufs=1) as npool,
            tc.tile_pool(name="ps", bufs=2, space="PSUM") as psp,
        ):
            wq_sb = cpool.tile([P, 8 * C], BF16)
            wk_sb = cpool.tile([P, 8 * C], BF16)
            wv_sb = cpool.tile([P, 8 * C], BF16)
            wp_sb = cpool.tile([P, 8 * C], BF16)
            qT_sb = cpool.tile([P, NP * 1024], BF16)
            kT_sb = cpool.tile([P, NP * T], BF16)
            v_sb = cpool.tile([P, 16 * NP * VW], BF16)
            masks_sb = cpool.tile([P, 16 * QC], BF16)
            bias_bc = cpool.tile([P, C], BF16)
            xq3 = cpool.tile([P, 8 * QC], BF16)  # slot-3 query cols of x

            attn_tiles = {}

            def get_attn(k):
                # per-slot attention buffer [128, p*256+t], ring of 2
                if k not in attn_tiles:
                    attn_tiles[k] = cpool.tile(
                        [P, NP * QC], BF16, tag="at", bufs=2, name=f"attn{k}"
                    )
                return attn_tiles[k]

            def dma_w(dst, src):
                # whole [C, C] weight -> [128, 8*C] SBUF in one DMA
                nc.sync.dma_start(
                    dst[:].rearrange("p (g c) -> p g c", c=C),
                    src.rearrange("(g p) c -> p g c", p=P),
                )

            def dma_x(xs, s):
                nc.sync.dma_start(
                    xs[:].rearrange("p (g c) -> p g c", c=512),
                    xT.rearrange("(g p) t -> p g t", p=P)[
                        :, :, s * 512:(s + 1) * 512],
                )

            def q_stage(s, xs, step=512, off=None):
                off = QOFF[s] if off is None else off
                for p in range(NP):
                    qp = psp.tile([P, QC], F32, tag="mm", name=f"qp{s}_{p}")
                    for g in range(8):
                        nc.tensor.matmul(
                            qp[:],
                            wq_sb[:, g * C + p * P:][:, :P],
                            xs[:, g * step + off:][:, :QC],
                            start=(g == 0), stop=(g == 7),
                        )
                    nc.scalar.copy(qT_sb[:, p * 1024 + s * QC:][:, :QC], qp[:])

            def k_group(s, xs, p):
                kp = psp.tile([P, 512], F32, tag="mm", name=f"kp{s}_{p}")
                for g in range(8):
                    nc.tensor.matmul(
                        kp[:],
                        wk_sb[:, g * C + p * P:][:, :P],
                        xs[:, g * 512:(g + 1) * 512],
                        start=(g == 0), stop=(g == 7),
                    )
                nc.vector.tensor_copy(kT_sb[:, p * T + s * 512:][:, :512], kp[:])

            def v_stage(s, xs):
                for jj in range(4):
                    j = 4 * s + jj
                    for hc in range(2):
                        vp = psp.tile([P, 512], F32, tag="mm",
                                      name=f"vp{j}_{hc}")
                        for g in range(8):
                            nc.tensor.matmul(
                                vp[:],
                                xs[:, g * 512 + jj * P:][:, :P],
                                wv_sb[:, g * C + hc * 512:][:, :512],
                                start=(g == 0), stop=(g == 7),
                            )
                        vdst = v_sb[:, j * (NP * VW) + hc * 4 * VW:][:, :4 * VW]
                        v3 = vdst.rearrange("p (l c) -> p l c", c=VW)
                        s3 = vp[:].rearrange("p (l c) -> p l c", c=P)
                        nc.scalar.copy(v3[:, :, 0:64], s3[:, :, 0:64])
                        nc.scalar.copy(v3[:, :, 65:129], s3[:, :, 64:128])

            def c_part(k, p, u_lo, u_hi, name):
                """Score/exp/mask/AV for key-tile pairs u_lo..u_hi-1 of
                (slot k, pair p). Returns the PSUM accumulator (one bank:
                one start=True on the first matmul, stop=True on the last
                of THIS part; interior matmuls overwrite-where-unset)."""
                pn = PNS[k]
                avp = psp.tile([65, 512], F32, tag="av", name=f"av{name}")
                qA = qT_sb[0:64, p * 1024 + k * QC:][:, :QC]
                qB = qT_sb[64:128, p * 1024 + k * QC:][:, :QC]
                pend = None

                def emit_av(pv):
                    e_t, j0 = pv
                    first = j0 == 2 * u_lo
                    last = j0 + 2 == 2 * u_hi
                    b0 = j0 * (NP * VW) + p * VW
                    b1 = (j0 + 1) * (NP * VW) + p * VW
                    nc.tensor.matmul(avp[:, 0:QC], v_sb[:, b0:b0 + 65],
                                     e_t[:, 0:QC], start=first, stop=False)
                    nc.tensor.matmul(avp[:, 0:QC], v_sb[:, b1:b1 + 65],
                                     e_t[:, QC:2 * QC],
                                     start=False, stop=False)
                    nc.tensor.matmul(avp[:, QC:2 * QC], v_sb[:, b0 + 65:b0 + VW],
                                     e_t[:, 2 * QC:3 * QC],
                                     start=False, stop=False)
                    nc.tensor.matmul(avp[:, QC:2 * QC], v_sb[:, b1 + 65:b1 + VW],
                                     e_t[:, 3 * QC:4 * QC],
                                     start=False, stop=last)

                for u in range(u_lo, u_hi):
                    j0 = 2 * u
                    kt0 = kT_sb[:, p * T + j0 * P:][:, :P]
                    kt1 = kT_sb[:, p * T + (j0 + 1) * P:][:, :P]
                    # sc spans 2 PSUM banks (cols 0:512 / 512:1024): one
                    # start=True per bank; the second matmul into a bank
                    # runs accumulate-mode and overwrites its untouched half.
                    sc = psp.tile([P, 4 * QC], F32, tag="sc",
                                  name=f"sc{name}_{u}")
                    nc.tensor.matmul(sc[:, 0:QC], kt0[0:64, :], qA,
                                     start=True, stop=False,
                                     tile_position=(0, 0))
                    nc.tensor.matmul(sc[:, 2 * QC:3 * QC], kt0[64:128, :], qB,
                                     start=True, stop=False,
                                     tile_position=(64, 0))
                    nc.tensor.matmul(sc[:, QC:2 * QC], kt1[0:64, :], qA,
                                     start=False, stop=True,
                                     tile_position=(0, 0))
                    nc.tensor.matmul(sc[:, 3 * QC:4 * QC], kt1[64:128, :], qB,
                                     start=False, stop=True,
                                     tile_position=(64, 0))
                    e_t = epool.tile([P, 4 * QC], BF16, tag="e",
                                     name=f"e{name}_{u}")
                    nc.scalar.activation(e_t[:], sc[:], EXP, scale=SCALE)
                    if u >= pn // 2 - 2:
                        l0 = j0 - (pn - 4)
                        mi = (k * 4 + l0) * QC
                        m2 = masks_sb[:, mi:mi + 2 * QC]
                        nc.vector.tensor_mul(e_t[:, 0:2 * QC],
                                             e_t[:, 0:2 * QC], m2)
                        nc.vector.tensor_mul(e_t[:, 2 * QC:4 * QC],
                                             e_t[:, 2 * QC:4 * QC], m2)
                    if pend is not None:
                        emit_av(pend)
                    pend = (e_t, j0)
                emit_av(pend)
                return avp

            def c_norm(k, p, avst):
                """Normalize staged [65, 512] bf16 AV into the attn ring."""
                rs = npool.tile([1, 2 * QC], F32, tag="rs", name=f"rs{k}_{p}")
                nc.vector.tensor_copy(rs[:], avst[64:65, :])
                rc = npool.tile([1, 2 * QC], F32, tag="rc", name=f"rc{k}_{p}")
                nc.vector.reciprocal_approx_fast(rc[:], rs[:])
                rb = npool.tile([64, 2 * QC], F32, tag="rb", name=f"rb{k}_{p}")
                nc.gpsimd.partition_broadcast(rb[:], rc[:])
                attn_r = get_attn(k)
                col = p * QC
                nc.vector.tensor_mul(attn_r[0:64, col:col + QC],
                                     avst[0:64, 0:QC], rb[:, 0:QC])
                nc.vector.tensor_mul(attn_r[64:128, col:col + QC],
                                     avst[0:64, QC:2 * QC], rb[:, QC:2 * QC])

            def c_run(k, p):
                avp = c_part(k, p, 0, PNS[k] // 2, f"{k}_{p}")
                avst = npool.tile([65, 512], BF16, tag="avst", bufs=2,
                                  name=f"avst{k}_{p}")
                nc.vector.tensor_copy(avst[:], avp[:])
                c_norm(k, p, avst)

            def proj(k):
                attn_r = get_attn(k)
                for tt in range(2):
                    for oc in range(2):
                        pp = psp.tile([P, 512], F32, tag="mm",
                                      name=f"pp{k}_{tt}_{oc}")
                        for g in range(NP):
                            nc.tensor.matmul(
                                pp[:],
                                attn_r[:, g * QC + tt * P:][:, :P],
                                wp_sb[:, g * C + oc * 512:][:, :512],
                                start=(g == 0), stop=(g == 7),
                            )
                        ot = opool.tile([P, 512], BF16, tag="ot",
                                        name=f"ot{k}_{tt}_{oc}")
                        nc.vector.tensor_add(
                            ot[:], pp[:], bias_bc[:, oc * 512:(oc + 1) * 512]
                        )
                        nc.sync.dma_start(
                            out[k, tt * P:(tt + 1) * P, oc * 512:(oc + 1) * 512],
                            ot[:],
                        )

            # startup: interleave x-stripe-0 and wq per-g DMAs so Q(0) can
            # begin after the first blocks land; bulk weights follow.
            xs0 = xpool.tile([P, 8 * 512], BF16, tag="xs", name="xs0")
            for g in range(8):
                nc.sync.dma_start(
                    xs0[:, g * 512:(g + 1) * 512],
                    xT[g * P:(g + 1) * P, 0:512],
                )
                nc.sync.dma_start(
                    wq_sb[:, g * C:(g + 1) * C], wq2[g * P:(g + 1) * P, :]
                )
            dma_w(wk_sb, wk2)
            dma_w(wv_sb, wv2)
            q_stage(0, xs0)
            for p in range(NP):
                k_group(0, xs0, p)
            dma_w(wp_sb, wp)
            nc.sync.dma_start(masks_sb[:], masks[:])
            nc.sync.dma_start(bias_bc[:], bias2[:])
            # slot-3 query columns of x (stripe 3, offset 256), prefetched
            nc.sync.dma_start(
                xq3[:].rearrange("p (g c) -> p g c", c=QC),
                xT.rearrange("(g p) t -> p g t", p=P)[
                    :, :, 3 * 512 + QOFF[3]:3 * 512 + QOFF[3] + QC],
            )
            # ones columns of v (col = 65*m + 64 for m in 0..255)
            vones = v_sb[:].rearrange("p (m o) -> p m o", o=65)[:, :, 64:65]
            nc.vector.memset(vones, 1.0)
            v_stage(0, xs0)

            av1 = {}

            def c3_partial(p, u_lo, u_hi, tag_first):
                avp = c_part(3, p, u_lo, u_hi, f"3{tag_first}_{p}")
                if tag_first == "a":
                    av1[p] = npool.tile([65, 512], BF16, tag="av1",
                                        bufs=8, name=f"av1_{p}")
                    nc.vector.tensor_copy(av1[p][:], avp[:])
                else:
                    nc.vector.tensor_add(av1[p][:], avp[:], av1[p][:])

            for s in range(1, 4):
                xs = xpool.tile([P, 8 * 512], BF16, tag="xs", name=f"xs{s}")
                dma_x(xs, s)
                if s < 3:
                    q_stage(s, xs)
                if s == 2:
                    q_stage(3, xq3, step=QC, off=0)
                for p in range(NP):
                    c_run(s - 1, p)
                    k_group(s, xs, p)
                    if s == 2:
                        c3_partial(p, 0, 4, "a")   # slot 3, key tiles 0-7
                    elif s == 3:
                        c3_partial(p, 4, 6, "b")   # slot 3, key tiles 8-11
                v_stage(s, xs)
                if s >= 2:
                    proj(s - 2)
            proj(2)
            for p in range(NP):
                # slot 3, key tiles 12-15 + merge with staged partial + norm
                avp2 = c_part(3, p, 6, 8, f"3c_{p}")
                avst = npool.tile([65, 512], BF16, tag="avst", bufs=2,
                                  name=f"avst3_{p}")
                nc.vector.tensor_add(avst[:], avp2[:], av1[p][:])
                c_norm(3, p, avst)
            proj(3)
    return nc


def _make_masks(half):
    m = np.zeros((P, 16 * QC), np.float32)
    s = np.arange(P)[:, None]
    t = np.arange(QC)[None, :]
    for k in range(4):
        q = SLOT_CHUNKS[half][k]
        pn = PNS[k]
        for l in range(4):
            j = pn - 4 + l
            a = j if half == 0 else 4 * (j // 4) + (j + 2) % 4
            m[:, (k * 4 + l) * QC:(k * 4 + l + 1) * QC] = (
                a * P + s <= q * QC + t
            )
    return m.astype(BD)


_CACHE = {}


def _get_nc():
    if "nc" not in _CACHE:
        nc = bacc.Bacc("TRN2", target_bir_lowering=False, debug=False)
        build_kernel(nc)
        nc.compile()
        _CACHE["nc"] = nc
    return _CACHE["nc"]


def make_in_maps(x, wq, wk, wv, w_proj, b_proj):
    x = np.asarray(x, np.float32)
    wq2 = np.ascontiguousarray(
        np.transpose(np.asarray(wq), (1, 0, 2)).reshape(C, C)).astype(BD)
    wk2 = np.ascontiguousarray(
        np.transpose(np.asarray(wk), (1, 0, 2)).reshape(C, C)).astype(BD)
    wv2 = np.ascontiguousarray(
        np.transpose(np.asarray(wv), (1, 0, 2)).reshape(C, C)).astype(BD)
    wpm = np.asarray(w_proj, np.float32).astype(BD)
    bias2 = np.tile(np.asarray(b_proj, np.float32).reshape(1, C), (P, 1))
    bias2 = np.ascontiguousarray(bias2).astype(BD)
    masks_h = [_make_masks(0), _make_masks(1)]

    in_maps = []
    for core in range(8):
        b, half = core // 2, core % 2
        xb = x[b]
        if half == 1:
            # swap the two 256-blocks of each 512-token stripe
            xb = xb.reshape(4, 2, QC, C)[:, ::-1].reshape(T, C)
        xTb = np.ascontiguousarray(xb.T).astype(BD)
        in_maps.append({
            "xT": xTb,
            "wq2": wq2, "wk2": wk2, "wv2": wv2,
            "wp": wpm, "bias2": bias2, "masks": masks_h[half],
        })
    return in_maps


def assemble(results):
    full = np.zeros((B, T, C), np.float32)
    for core in range(8):
        b, half = core // 2, core % 2
        o = np.asarray(results[core]["out"], dtype=np.float32)
        for k, q in enumerate(SLOT_CHUNKS[half]):
            full[b, q * QC:(q + 1) * QC] = o[k]
    return full


def kernel(x, wq, wk, wv, w_proj, b_proj, _trace=False, _tmpdir=None):
    in_maps = make_in_maps(x, wq, wk, wv, w_proj, b_proj)
    nc = _get_nc()
    res = run_bass_kernel_spmd(
        nc, in_maps, core_ids=list(range(8)), trace=_trace, tmpdir=_tmpdir
    )
    if _trace:
        _CACHE["last_result"] = res
    return assemble(res.results)
